# revision 9
# baseline (speedup 1.0000x reference)
"""Trainium2 Bass kernel for a dense transformer decoder layer.

Sharding: token-parallel across 8 cores. Core c handles batch b=c//2,
sequence half h=c%2 (512 query tokens). Each core recomputes K/V for its
batch's full 1024-token sequence (cheap) so no collectives are needed.

All activations live in transposed [feature, token] layout so every matmul
contraction sits on the partition axis. Matmuls run in float32r (full PE
speed at N>=256, ~1.6e-4 relative error). Cross-partition reductions
(rms-norm sums, softmax denominators) are done with ones-vector matmuls on
the PE. Rotary embedding is applied as qn*cosA + (P@qn)*sinA where P is a
+-1 permutation matmul; the (1+norm_w) and 1/sqrt(HD) factors are folded
into host-precomputed cos/sin tables, and (1+ln_w) into the weights.
Softmax skips max-subtraction (rms-normed q/k bound scores to ~13, safely
inside fp32 exp range); the causal mask is an exp-bias column for whole
blocks plus a 0/1 multiply for the 4 triangular local blocks.
"""

import numpy as np

import concourse.bass as bass
import concourse.tile as tile
from concourse import bacc, mybir
from concourse.bass_utils import run_bass_kernel_spmd

B, S, H = 4, 1024, 2048
NH, NKV, HD = 16, 4, 128
FF = 8192
EPS = 1e-6
P = 128
T = 512            # local query tokens per core
HT = H // P        # 16 hidden tiles
FT = FF // P       # 64 ff tiles
NKB = S // P       # 8 key blocks
NCORES = 8

F32 = mybir.dt.float32
F32R = mybir.dt.float32r
BF16 = mybir.dt.bfloat16
AF = mybir.ActivationFunctionType

_BUILD_CACHE = {}


def _build_program():
    if "nc" in _BUILD_CACHE:
        return _BUILD_CACHE["nc"]

    nc = bacc.Bacc("TRN2", target_bir_lowering=False, debug=False,
                   num_devices=NCORES)

    # ---- DRAM I/O ----
    xt_d = nc.dram_tensor("xt", [H, S], F32, kind="ExternalInput")
    wq_d = nc.dram_tensor("wq", [NH, P, HT, P], F32R, kind="ExternalInput")
    wk_d = nc.dram_tensor("wk", [NKV, P, HT, P], F32R, kind="ExternalInput")
    wv_d = nc.dram_tensor("wv", [HT, P, NKV * HD], F32R, kind="ExternalInput")
    wz_d = nc.dram_tensor("wz", [NH, P, HT, P], F32R, kind="ExternalInput")
    wo_d = nc.dram_tensor("wo", [HT, P, NH, P], F32R, kind="ExternalInput")
    wg_d = nc.dram_tensor("wg", [FT, P, HT, P], F32R, kind="ExternalInput")
    wu_d = nc.dram_tensor("wu", [FT, P, HT, P], F32R, kind="ExternalInput")
    wd_d = nc.dram_tensor("wd", [HT, P, FT, P], F32R, kind="ExternalInput")
    cosq_d = nc.dram_tensor("cosq", [P, T], F32, kind="ExternalInput")
    sinq_d = nc.dram_tensor("sinq", [P, T], F32, kind="ExternalInput")
    cosk_d = nc.dram_tensor("cosk", [P, S], F32, kind="ExternalInput")
    sink_d = nc.dram_tensor("sink", [P, S], F32, kind="ExternalInput")
    maskl_d = nc.dram_tensor("maskl", [P, 4, T], F32, kind="ExternalInput")
    biasr_d = nc.dram_tensor("biasr", [P, 4], F32, kind="ExternalInput")
    ones_d = nc.dram_tensor("ones", [P, P], F32R, kind="ExternalInput")
    rotp_d = nc.dram_tensor("rotp", [P, P], F32R, kind="ExternalInput")
    out_d = nc.dram_tensor("outT", [H, T], F32, kind="ExternalOutput")
    x2_d = nc.dram_tensor("x2scratch", [H, T], F32)   # internal scratch

    ts = bass.ts

    with tile.TileContext(nc) as tc:
        with tc.tile_pool(name="consts", bufs=1) as cpool:
            ones_t = cpool.tile([P, P], F32R, name="ones")
            nc.sync.dma_start(ones_t[:], ones_d[:])
            rotp_t = cpool.tile([P, P], F32R, name="rotp")
            nc.sync.dma_start(rotp_t[:], rotp_d[:])
            eps_t = cpool.tile([P, 1], F32, name="eps")
            nc.vector.memset(eps_t[:], EPS)

            # ============ attention half: phases A-D ============
            with tc.tile_pool(name="qr", bufs=NH) as qr_pool, \
                 tc.tile_pool(name="kr", bufs=NKV) as kr_pool, \
                 tc.tile_pool(name="vv", bufs=NKB) as v_pool, \
                 tc.tile_pool(name="sz", bufs=NH) as sz_pool:

                qr_t = [qr_pool.tile([P, T], F32R, name="qr")
                        for _ in range(NH)]
                kr_t = [kr_pool.tile([P, S], F32R, name="kr")
                        for _ in range(NKV)]
                v_t = [v_pool.tile([P, NKV * HD], F32R, name="vv")
                       for _ in range(NKB)]
                sz_t = [sz_pool.tile([P, T], BF16, name="sz")
                        for _ in range(NH)]

                # ---- Phase A+B: input rmsnorm + QKVZ projections ----
                with tc.tile_pool(name="tabs", bufs=1) as tabs, \
                     tc.tile_pool(name="xa", bufs=3) as xa_pool, \
                     tc.tile_pool(name="xb", bufs=3) as xb_pool, \
                     tc.tile_pool(name="atmp", bufs=2) as atmp, \
                     tc.tile_pool(name="wstr", bufs=2) as wstr, \
                     tc.tile_pool(name="wvstr", bufs=3) as wvstr, \
                     tc.tile_pool(name="btmp", bufs=2) as btmp, \
                     tc.tile_pool(name="psA", bufs=2, space="PSUM") as psA, \
                     tc.tile_pool(name="psV", bufs=4, space="PSUM") as psV, \
                     tc.tile_pool(name="psS", bufs=1, space="PSUM") as psS, \
                     tc.tile_pool(name="psR", bufs=1, space="PSUM") as psR:

                    cosq_t = tabs.tile([P, T], F32, name="cosq")
                    nc.sync.dma_start(cosq_t[:], cosq_d[:])
                    sinq_t = tabs.tile([P, T], F32, name="sinq")
                    nc.sync.dma_start(sinq_t[:], sinq_d[:])
                    cosk_t = tabs.tile([P, S], F32, name="cosk")
                    nc.sync.dma_start(cosk_t[:], cosk_d[:])
                    sink_t = tabs.tile([P, S], F32, name="sink")
                    nc.sync.dma_start(sink_t[:], sink_d[:])

                    def rms_xn(c, xn_pool):
                        # pass 1: sum of squares; pass 2: x * rstd
                        xn_c = []
                        ps = psS.tile([P, T], F32, name="ssqx")
                        for h in range(HT):
                            xf = xa_pool.tile([P, T], F32, name="xa")
                            nc.sync.dma_start(xf[:], xt_d[ts(h, P), ts(c, T)])
                            xsq = atmp.tile([P, T], F32R, name="xsq")
                            nc.scalar.activation(xsq[:], xf[:], AF.Square)
                            nc.tensor.matmul(ps[:], ones_t[:], xsq[:],
                                             start=(h == 0),
                                             stop=(h == HT - 1))
                        sq = atmp.tile([P, T], F32, name="sq")
                        nc.scalar.activation(sq[:], ps[:], AF.Sqrt,
                                             scale=1.0 / H, bias=eps_t[:])
                        rstd = atmp.tile([P, T], F32, name="rstd")
                        nc.vector.reciprocal(rstd[:], sq[:])
                        for h in range(HT):
                            xf2 = xb_pool.tile([P, T], F32, name="xb")
                            nc.sync.dma_start(xf2[:],
                                              xt_d[ts(h, P), ts(c, T)])
                            xn = xn_pool.tile([P, T], F32R, name="xn")
                            nc.vector.tensor_mul(xn[:], xf2[:], rstd[:])
                            xn_c.append(xn)
                        return xn_c

                    def v_proj(c, xn_c):
                        psv = [psV.tile([P, NKV * HD], F32, name="vps")
                               for _ in range(4)]
                        for h in range(HT):
                            wvt = wvstr.tile([P, NKV * HD], F32R, name="wv")
                            nc.sync.dma_start(wvt[:], wv_d[h])
                            for tb in range(4):
                                nc.tensor.matmul(
                                    psv[tb][:],
                                    xn_c[h][:, ts(tb, P)], wvt[:],
                                    start=(h == 0), stop=(h == HT - 1))
                        for tb in range(4):
                            nc.scalar.copy(v_t[c * 4 + tb][:], psv[tb][:])

                    def qk_pipeline(ps, out_tile, cos_ap, sin_ap):
                        # per-head rmsnorm ((1+w) in tables) + rope
                        q2 = btmp.tile([P, T], F32R, name="q2")
                        nc.scalar.activation(q2[:], ps[:], AF.Square)
                        ps2 = psS.tile([P, T], F32, name="ssqx")
                        nc.tensor.matmul(ps2[:], ones_t[:], q2[:],
                                         start=True, stop=True)
                        sq = btmp.tile([P, T], F32, name="sqq")
                        nc.scalar.activation(sq[:], ps2[:], AF.Sqrt,
                                             scale=1.0 / HD, bias=eps_t[:])
                        rq = btmp.tile([P, T], F32, name="rqq")
                        nc.vector.reciprocal(rq[:], sq[:])
                        qn = btmp.tile([P, T], F32R, name="qn")
                        nc.vector.tensor_mul(qn[:], ps[:], rq[:])
                        psr = psR.tile([P, T], F32, name="rot")
                        nc.tensor.matmul(psr[:], rotp_t[:], qn[:],
                                         start=True, stop=True)
                        t1 = btmp.tile([P, T], F32, name="t1")
                        nc.gpsimd.tensor_mul(t1[:], qn[:], cos_ap)
                        t2 = btmp.tile([P, T], F32, name="t2")
                        nc.vector.tensor_mul(t2[:], psr[:], sin_ap)
                        nc.vector.tensor_add(out_tile, t1[:], t2[:])

                    def proj_ps(w_dram, o, xn_c):
                        wgt = wstr.tile([P, HT, P], F32R, name="wqg")
                        nc.sync.dma_start(wgt[:], w_dram[o])
                        ps = psA.tile([P, T], F32, name="proj")
                        for h in range(HT):
                            nc.tensor.matmul(ps[:], wgt[:, h, :], xn_c[h][:],
                                             start=(h == 0),
                                             stop=(h == HT - 1))
                        return ps

                    # chunk 0: local tokens (q, z, k half, v half)
                    with tc.tile_pool(name="xn0", bufs=HT) as xn0:
                        xn_c = rms_xn(0, xn0)
                        v_proj(0, xn_c)
                        for o in range(NH):
                            ps = proj_ps(wq_d, o, xn_c)
                            qk_pipeline(ps[:], qr_t[o][:],
                                        cosq_t[:], sinq_t[:])
                        for kv in range(NKV):
                            ps = proj_ps(wk_d, kv, xn_c)
                            qk_pipeline(ps[:], kr_t[kv][:, ts(0, T)],
                                        cosk_t[:, ts(0, T)],
                                        sink_t[:, ts(0, T)])
                        for o in range(NH):
                            ps = proj_ps(wz_d, o, xn_c)
                            nc.scalar.activation(sz_t[o][:], ps[:],
                                                 AF.Sigmoid)

                    # chunk 1: remote-half tokens (k and v only)
                    with tc.tile_pool(name="xn1", bufs=HT) as xn1:
                        xn_c = rms_xn(1, xn1)
                        v_proj(1, xn_c)
                        for kv in range(NKV):
                            ps = proj_ps(wk_d, kv, xn_c)
                            qk_pipeline(ps[:], kr_t[kv][:, ts(1, T)],
                                        cosk_t[:, ts(1, T)],
                                        sink_t[:, ts(1, T)])

                # ---- Phase C: attention ----
                with tc.tile_pool(name="gated", bufs=NH) as gpool:
                    gated_t = []
                    with tc.tile_pool(name="mask", bufs=1) as mpool, \
                         tc.tile_pool(name="probs", bufs=4) as ppool, \
                         tc.tile_pool(name="ctmp", bufs=2) as ctmp, \
                         tc.tile_pool(name="psSc", bufs=2,
                                      space="PSUM") as psSc, \
                         tc.tile_pool(name="psAt", bufs=2,
                                      space="PSUM") as psAt, \
                         tc.tile_pool(name="psSm", bufs=2,
                                      space="PSUM") as psSm, \
                         tc.tile_pool(name="psBc", bufs=2,
                                      space="PSUM") as psBc:

                        maskl_t = mpool.tile([P, 4, T], F32, name="maskl")
                        nc.sync.dma_start(maskl_t[:], maskl_d[:])
                        biasr_t = mpool.tile([P, 4], F32, name="biasr")
                        nc.sync.dma_start(biasr_t[:], biasr_d[:])

                        for o in range(NH):
                            kv = o // NKV
                            ps_att = psAt.tile([P, T], F32, name="att")
                            ps_sum = psSm.tile([1, T], F32, name="sum")
                            for j in range(NKB):
                                ps_sc = psSc.tile([P, T], F32, name="sc")
                                nc.tensor.matmul(ps_sc[:],
                                                 kr_t[kv][:, ts(j, P)],
                                                 qr_t[o][:],
                                                 start=True, stop=True)
                                probs = ppool.tile([P, T], F32R,
                                                   name="probs")
                                if j < 4:
                                    nc.scalar.activation(probs[:], ps_sc[:],
                                                         AF.Exp)
                                    nc.vector.tensor_mul(probs[:], probs[:],
                                                         maskl_t[:, j, :])
                                else:
                                    nc.scalar.activation(
                                        probs[:], ps_sc[:], AF.Exp,
                                        bias=biasr_t[:, ts(j - 4, 1)])
                                nc.tensor.matmul(ps_att[:],
                                                 v_t[j][:, ts(kv, P)],
                                                 probs[:], start=(j == 0),
                                                 stop=(j == NKB - 1))
                                nc.tensor.matmul(ps_sum[:], ones_t[:, 0:1],
                                                 probs[:], start=(j == 0),
                                                 stop=(j == NKB - 1))
                            rec = ctmp.tile([1, T], F32R, name="rec")
                            with nc.allow_low_precision("f32r = f32 bits"):
                                nc.vector.reciprocal(rec[:], ps_sum[:])
                            ps_bc = psBc.tile([P, T], F32, name="bc")
                            nc.tensor.matmul(ps_bc[:], ones_t[0:1, :],
                                             rec[:], start=True, stop=True)
                            recrep = ctmp.tile([P, T], F32, name="recrep")
                            nc.scalar.copy(recrep[:], ps_bc[:])
                            t1 = ctmp.tile([P, T], F32, name="ct1")
                            nc.vector.tensor_mul(t1[:], ps_att[:],
                                                 recrep[:])
                            g = gpool.tile([P, T], F32R, name="gated")
                            nc.vector.tensor_mul(g[:], t1[:], sz_t[o][:])
                            gated_t.append(g)

                    # ---- Phase D: o_proj + residual -> x2 scratch ----
                    with tc.tile_pool(name="wostr", bufs=2) as wostr, \
                         tc.tile_pool(name="rtmp", bufs=2) as rtmp, \
                         tc.tile_pool(name="x2w", bufs=3) as x2w, \
                         tc.tile_pool(name="psO", bufs=2,
                                      space="PSUM") as psO:
                        for hp in range(HT):
                            wgt = wostr.tile([P, NH, P], F32R, name="wog")
                            nc.sync.dma_start(wgt[:], wo_d[hp])
                            ps = psO.tile([P, T], F32, name="ops")
                            for o in range(NH):
                                nc.tensor.matmul(ps[:], wgt[:, o, :],
                                                 gated_t[o][:],
                                                 start=(o == 0),
                                                 stop=(o == NH - 1))
                            rx = rtmp.tile([P, T], F32, name="resid")
                            nc.sync.dma_start(rx[:], xt_d[ts(hp, P), 0:T])
                            x2t = x2w.tile([P, T], F32, name="x2t")
                            nc.vector.tensor_add(x2t[:], ps[:], rx[:])
                            nc.sync.dma_start(x2_d[ts(hp, P), :], x2t[:])

            # ============ MLP half: phases E-G ============
            with tc.tile_pool(name="mm", bufs=FT // 2) as mpool2, \
                 tc.tile_pool(name="h2", bufs=HT) as h2_pool:

                h2_t = [h2_pool.tile([P, T], F32R, name="h2")
                        for _ in range(HT)]

                # ---- Phase E: post rmsnorm (x2 from scratch DRAM) ----
                with tc.tile_pool(name="ea", bufs=3) as ea_pool, \
                     tc.tile_pool(name="eb", bufs=3) as eb_pool, \
                     tc.tile_pool(name="etmp", bufs=2) as etmp, \
                     tc.tile_pool(name="psE", bufs=1, space="PSUM") as psE:
                    ps = psE.tile([P, T], F32, name="essq")
                    for h in range(HT):
                        xf = ea_pool.tile([P, T], F32, name="ea")
                        nc.sync.dma_start(xf[:], x2_d[ts(h, P), :])
                        xsq = etmp.tile([P, T], F32R, name="exsq")
                        nc.scalar.activation(xsq[:], xf[:], AF.Square)
                        nc.tensor.matmul(ps[:], ones_t[:], xsq[:],
                                         start=(h == 0), stop=(h == HT - 1))
                    sq = etmp.tile([P, T], F32, name="esq")
                    nc.scalar.activation(sq[:], ps[:], AF.Sqrt,
                                         scale=1.0 / H, bias=eps_t[:])
                    rstd = etmp.tile([P, T], F32, name="erstd")
                    nc.vector.reciprocal(rstd[:], sq[:])
                    for h in range(HT):
                        xf2 = eb_pool.tile([P, T], F32, name="eb")
                        nc.sync.dma_start(xf2[:], x2_d[ts(h, P), :])
                        nc.vector.tensor_mul(h2_t[h][:], xf2[:], rstd[:])

                # ---- Phase F+G: gate/up/silu/down in two f-halves ----
                FH = FT // 2
                with tc.tile_pool(name="wgstr", bufs=2) as wgstr, \
                     tc.tile_pool(name="wustr", bufs=2) as wustr, \
                     tc.tile_pool(name="wdstr", bufs=2) as wdstr, \
                     tc.tile_pool(name="ftmp", bufs=2) as ftmp, \
                     tc.tile_pool(name="gtmp", bufs=3) as gtmp, \
                     tc.tile_pool(name="psG", bufs=2, space="PSUM") as psG, \
                     tc.tile_pool(name="psU", bufs=2, space="PSUM") as psU, \
                     tc.tile_pool(name="psD", bufs=2, space="PSUM") as psD:
                    for fh in range(2):
                        m_t = []
                        for fi in range(FH):
                            f = fh * FH + fi
                            wgt = wgstr.tile([P, HT, P], F32R, name="wgg")
                            nc.sync.dma_start(wgt[:], wg_d[f])
                            wut = wustr.tile([P, HT, P], F32R, name="wug")
                            nc.sync.dma_start(wut[:], wu_d[f])
                            psg = psG.tile([P, T], F32, name="gps")
                            psu = psU.tile([P, T], F32, name="ups")
                            for h in range(HT):
                                nc.tensor.matmul(psg[:], wgt[:, h, :],
                                                 h2_t[h][:], start=(h == 0),
                                                 stop=(h == HT - 1))
                            for h in range(HT):
                                nc.tensor.matmul(psu[:], wut[:, h, :],
                                                 h2_t[h][:], start=(h == 0),
                                                 stop=(h == HT - 1))
                            sg0 = ftmp.tile([P, T], F32, name="sgm")
                            nc.scalar.activation(sg0[:], psg[:], AF.Sigmoid)
                            sg = ftmp.tile([P, T], F32, name="silu")
                            nc.vector.tensor_mul(sg[:], psg[:], sg0[:])
                            mt = mpool2.tile([P, T], F32R, name="mt")
                            nc.vector.tensor_mul(mt[:], psu[:], sg[:])
                            m_t.append(mt)
                        # down projection partial over this f-half
                        for h in range(HT):
                            ps = psD.tile([P, T], F32, name="dps")
                            for q4 in range(2):
                                wdt = wdstr.tile([P, FH // 2, P], F32R,
                                                 name="wdg")
                                nc.sync.dma_start(
                                    wdt[:],
                                    wd_d[h, :,
                                         ts(fh * 2 + q4, FH // 2), :])
                                for fi in range(FH // 2):
                                    fidx = q4 * (FH // 2) + fi
                                    nc.tensor.matmul(
                                        ps[:], wdt[:, fi, :], m_t[fidx][:],
                                        start=(fidx == 0),
                                        stop=(fidx == FH - 1))
                            prev = gtmp.tile([P, T], F32, name="gprev")
                            nc.sync.dma_start(
                                prev[:],
                                x2_d[ts(h, P), :] if fh == 0
                                else out_d[ts(h, P), :])
                            outt = gtmp.tile([P, T], F32, name="gout")
                            nc.vector.tensor_add(outt[:], ps[:], prev[:])
                            nc.sync.dma_start(out_d[ts(h, P), :], outt[:])

    nc.compile()
    _BUILD_CACHE["nc"] = nc
    return nc


def _prep_core_inputs(inputs):
    """Host-side preprocessing: fold norms/scales into weights and tables,
    transpose + tile weights for contiguous DMA, build per-core in_maps."""
    f32 = np.float32
    x = np.asarray(inputs["x"], f32)
    in_ln_w = np.asarray(inputs["in_ln_w"], f32)
    post_ln_w = np.asarray(inputs["post_ln_w"], f32)
    qn_w = np.asarray(inputs["qn_w"], f32)
    kn_w = np.asarray(inputs["kn_w"], f32)

    s_in = (1.0 + in_ln_w)[:, None]       # [H, 1] scale on contraction dim
    s_post = (1.0 + post_ln_w)[:, None]

    def tile_lhsT(wT, n_out_tiles):
        # wT: [K_total, M_total] -> [o_tile, p(=K within), i(=K tile), c]
        kt = wT.shape[0] // P
        a = np.ascontiguousarray(
            wT.reshape(kt, P, n_out_tiles, P).transpose(2, 1, 0, 3))
        return a.astype(f32)

    wq = tile_lhsT(np.asarray(inputs["Wq"], f32).T * s_in, NH)
    wk = tile_lhsT(np.asarray(inputs["Wk"], f32).T * s_in, NKV)
    wz = tile_lhsT(np.asarray(inputs["Wz"], f32).T * s_in, NH)
    wo = tile_lhsT(np.asarray(inputs["Wo"], f32).T, HT)
    wg = tile_lhsT(np.asarray(inputs["Wg"], f32).T * s_post, FT)
    wu = tile_lhsT(np.asarray(inputs["Wu"], f32).T * s_post, FT)
    wd = tile_lhsT(np.asarray(inputs["Wd"], f32).T, HT)
    wv = np.ascontiguousarray(
        (np.asarray(inputs["Wv"], f32).T * s_in).reshape(HT, P, NKV * HD)
    ).astype(f32)

    # rope tables
    inv_freq = 1.0 / (10000.0 ** (np.arange(0, HD, 2, dtype=f32) / HD))
    t = np.arange(S, dtype=f32)
    freqs = t[:, None] * inv_freq[None, :]
    emb = np.concatenate([freqs, freqs], axis=-1)     # [S, HD]
    cos_all, sin_all = np.cos(emb), np.sin(emb)
    rolled_q = np.roll(1.0 + qn_w, -64)
    rolled_k = np.roll(1.0 + kn_w, -64)
    inv_sqrt_hd = 1.0 / np.sqrt(np.float32(HD))

    ones = np.ones((P, P), f32)
    rotp = np.zeros((P, P), f32)
    for i in range(64):
        rotp[i + 64, i] = -1.0
        rotp[i, i + 64] = 1.0

    qk = np.arange(T)[None, :]            # query col
    kk = np.arange(P)[:, None]            # key row within block
    maskl = np.zeros((P, 4, T), f32)
    for j in range(4):
        maskl[:, j, :] = (P * j + kk <= qk).astype(f32)

    in_maps = []
    for c in range(NCORES):
        b, half = c // 2, c % 2
        p0 = half * T
        pos = np.concatenate([np.arange(p0, p0 + T),
                              np.arange(T - p0, 2 * T - p0)])  # local first
        xt = np.ascontiguousarray(x[b][pos].T)                 # [H, S]
        pos_q = pos[:T]
        cosq = np.ascontiguousarray(
            (cos_all[pos_q] * (1.0 + qn_w)[None, :] * inv_sqrt_hd).T)
        sinq = np.ascontiguousarray(
            (sin_all[pos_q] * rolled_q[None, :] * inv_sqrt_hd).T)
        cosk = np.ascontiguousarray((cos_all[pos] * (1.0 + kn_w)[None, :]).T)
        sink = np.ascontiguousarray((sin_all[pos] * rolled_k[None, :]).T)
        biasr = np.full((P, 4), 0.0 if half == 1 else -1e30, f32)
        in_maps.append({
            "xt": xt, "wq": wq, "wk": wk, "wv": wv, "wz": wz, "wo": wo,
            "wg": wg, "wu": wu, "wd": wd,
            "cosq": cosq, "sinq": sinq, "cosk": cosk, "sink": sink,
            "maskl": maskl, "biasr": biasr, "ones": ones, "rotp": rotp,
        })
    return in_maps


def kernel(**inputs):
    nc = _build_program()
    in_maps = _prep_core_inputs(inputs)
    res = run_bass_kernel_spmd(nc, in_maps, list(range(NCORES)))
    out = np.empty((B, S, H), np.float32)
    for c in range(NCORES):
        b, half = c // 2, c % 2
        out[b, half * T:(half + 1) * T, :] = res.results[c]["outT"].T
    return out


# revision 10
# speedup vs baseline: 73.9538x; 73.9538x over previous
"""Trainium2 Bass kernel for a dense transformer decoder layer.

Sharding: token-parallel across 8 cores. Core c handles batch b=c//2,
sequence half h=c%2 (512 query tokens). Each core recomputes K/V for its
batch's full 1024-token sequence (cheap) so no collectives are needed.

All activations live in transposed [feature, token] layout so every matmul
contraction sits on the partition axis. Matmuls run in float32r (full PE
speed at N>=256, ~1.6e-4 relative error). Cross-partition reductions
(rms-norm sums, softmax denominators) are done with ones-vector matmuls on
the PE. Rotary embedding is applied as qn*cosA + (P@qn)*sinA where P is a
+-1 permutation matmul; the (1+norm_w) and 1/sqrt(HD) factors are folded
into host-precomputed cos/sin tables, and (1+ln_w) into the weights.
Softmax skips max-subtraction (rms-normed q/k bound scores to ~13, safely
inside fp32 exp range); the causal mask is an exp-bias column for whole
blocks plus a 0/1 multiply for the 4 triangular local blocks.
"""

import numpy as np

import concourse.bass as bass
import concourse.tile as tile
from concourse import bacc, mybir
from concourse.bass_utils import run_bass_kernel_spmd

B, S, H = 4, 1024, 2048
NH, NKV, HD = 16, 4, 128
FF = 8192
EPS = 1e-6
P = 128
T = 512            # local query tokens per core
HT = H // P        # 16 hidden tiles
FT = FF // P       # 64 ff tiles
NKB = S // P       # 8 key blocks
NCORES = 8

F32 = mybir.dt.float32
F32R = mybir.dt.float32r
BF16 = mybir.dt.bfloat16
F16 = mybir.dt.float16
AF = mybir.ActivationFunctionType

_BUILD_CACHE = {}


def _build_program():
    if "nc" in _BUILD_CACHE:
        return _BUILD_CACHE["nc"]

    nc = bacc.Bacc("TRN2", target_bir_lowering=False, debug=False,
                   num_devices=NCORES)

    # ---- DRAM I/O ----
    xt_d = nc.dram_tensor("xt", [H, S], F32, kind="ExternalInput")
    wq_d = nc.dram_tensor("wq", [NH, P, HT, P], F32R, kind="ExternalInput")
    wk_d = nc.dram_tensor("wk", [NKV, P, HT, P], F32R, kind="ExternalInput")
    wv_d = nc.dram_tensor("wv", [HT, P, NKV * HD], F32R, kind="ExternalInput")
    wz_d = nc.dram_tensor("wz", [NH, P, HT, P], F32R, kind="ExternalInput")
    wo_d = nc.dram_tensor("wo", [HT, P, NH, P], F32R, kind="ExternalInput")
    wg_d = nc.dram_tensor("wg", [FT, P, HT, P], F32R, kind="ExternalInput")
    wu_d = nc.dram_tensor("wu", [FT, P, HT, P], F32R, kind="ExternalInput")
    wd_d = nc.dram_tensor("wd", [HT, P, FT, P], F32R, kind="ExternalInput")
    cosq_d = nc.dram_tensor("cosq", [P, T], F32, kind="ExternalInput")
    sinq_d = nc.dram_tensor("sinq", [P, T], F32, kind="ExternalInput")
    cosk_d = nc.dram_tensor("cosk", [P, S], F32, kind="ExternalInput")
    sink_d = nc.dram_tensor("sink", [P, S], F32, kind="ExternalInput")
    maskl_d = nc.dram_tensor("maskl", [P, 4, T], F32, kind="ExternalInput")
    biasr_d = nc.dram_tensor("biasr", [P, 4], F32, kind="ExternalInput")
    ones_d = nc.dram_tensor("ones", [P, P], F32R, kind="ExternalInput")
    rotp_d = nc.dram_tensor("rotp", [P, P], F32R, kind="ExternalInput")
    out_d = nc.dram_tensor("outT", [H, T], F32, kind="ExternalOutput")
    x2_d = nc.dram_tensor("x2scratch", [H, T], F32)   # internal scratch

    ts = bass.ts

    with tile.TileContext(nc) as tc:
        with tc.tile_pool(name="consts", bufs=1) as cpool:
            ones_t = cpool.tile([P, P], F32R, name="ones")
            nc.sync.dma_start(ones_t[:], ones_d[:])
            rotp_t = cpool.tile([P, P], F32R, name="rotp")
            nc.sync.dma_start(rotp_t[:], rotp_d[:])
            eps_t = cpool.tile([P, 1], F32, name="eps")
            nc.vector.memset(eps_t[:], EPS)

            # ============ attention half: phases A-D ============
            with tc.tile_pool(name="qr", bufs=NH) as qr_pool, \
                 tc.tile_pool(name="kr", bufs=NKV) as kr_pool, \
                 tc.tile_pool(name="vv", bufs=NKB) as v_pool, \
                 tc.tile_pool(name="sz", bufs=NH) as sz_pool:

                qr_t = [qr_pool.tile([P, T], F32R, name="qr")
                        for _ in range(NH)]
                kr_t = [kr_pool.tile([P, S], F32R, name="kr")
                        for _ in range(NKV)]
                v_t = [v_pool.tile([P, NKV * HD], F32R, name="vv")
                       for _ in range(NKB)]
                sz_t = [sz_pool.tile([P, T], F16, name="sz")
                        for _ in range(NH)]

                # ---- Phase A+B: input rmsnorm + QKVZ projections ----
                with tc.tile_pool(name="tabs", bufs=1) as tabs, \
                     tc.tile_pool(name="xa", bufs=3) as xa_pool, \
                     tc.tile_pool(name="xb", bufs=3) as xb_pool, \
                     tc.tile_pool(name="atmp", bufs=2) as atmp, \
                     tc.tile_pool(name="wstr", bufs=2) as wstr, \
                     tc.tile_pool(name="wvstr", bufs=3) as wvstr, \
                     tc.tile_pool(name="btmp", bufs=2) as btmp, \
                     tc.tile_pool(name="psA", bufs=2, space="PSUM") as psA, \
                     tc.tile_pool(name="psV", bufs=4, space="PSUM") as psV, \
                     tc.tile_pool(name="psS", bufs=1, space="PSUM") as psS, \
                     tc.tile_pool(name="psR", bufs=1, space="PSUM") as psR:

                    cosq_t = tabs.tile([P, T], F32, name="cosq")
                    nc.sync.dma_start(cosq_t[:], cosq_d[:])
                    sinq_t = tabs.tile([P, T], F32, name="sinq")
                    nc.sync.dma_start(sinq_t[:], sinq_d[:])
                    cosk_t = tabs.tile([P, S], F32, name="cosk")
                    nc.sync.dma_start(cosk_t[:], cosk_d[:])
                    sink_t = tabs.tile([P, S], F32, name="sink")
                    nc.sync.dma_start(sink_t[:], sink_d[:])

                    def rms_xn(c, xn_pool):
                        # pass 1: sum of squares; pass 2: x * rstd
                        xn_c = []
                        ps = psS.tile([P, T], F32, name="ssqx")
                        for h in range(HT):
                            xf = xa_pool.tile([P, T], F32, name="xa")
                            nc.sync.dma_start(xf[:], xt_d[ts(h, P), ts(c, T)])
                            xsq = atmp.tile([P, T], F32R, name="xsq")
                            nc.scalar.activation(xsq[:], xf[:], AF.Square)
                            nc.tensor.matmul(ps[:], ones_t[:], xsq[:],
                                             start=(h == 0),
                                             stop=(h == HT - 1))
                        sq = atmp.tile([P, T], F32, name="sq")
                        nc.scalar.activation(sq[:], ps[:], AF.Sqrt,
                                             scale=1.0 / H, bias=eps_t[:])
                        rstd = atmp.tile([P, T], F32, name="rstd")
                        nc.vector.reciprocal(rstd[:], sq[:])
                        for h in range(HT):
                            xf2 = xb_pool.tile([P, T], F32, name="xb")
                            nc.sync.dma_start(xf2[:],
                                              xt_d[ts(h, P), ts(c, T)])
                            xn = xn_pool.tile([P, T], F32R, name="xn")
                            nc.vector.tensor_mul(xn[:], xf2[:], rstd[:])
                            xn_c.append(xn)
                        return xn_c

                    def v_proj(c, xn_c):
                        psv = [psV.tile([P, NKV * HD], F32, name="vps")
                               for _ in range(4)]
                        for h in range(HT):
                            wvt = wvstr.tile([P, NKV * HD], F32R, name="wv")
                            nc.sync.dma_start(wvt[:], wv_d[h])
                            for tb in range(4):
                                nc.tensor.matmul(
                                    psv[tb][:],
                                    xn_c[h][:, ts(tb, P)], wvt[:],
                                    start=(h == 0), stop=(h == HT - 1))
                        for tb in range(4):
                            nc.scalar.copy(v_t[c * 4 + tb][:], psv[tb][:])

                    def qk_pipeline(ps, out_tile, cos_ap, sin_ap):
                        # per-head rmsnorm ((1+w) in tables) + rope
                        q2 = btmp.tile([P, T], F32R, name="q2")
                        nc.scalar.activation(q2[:], ps[:], AF.Square)
                        ps2 = psS.tile([P, T], F32, name="ssqx")
                        nc.tensor.matmul(ps2[:], ones_t[:], q2[:],
                                         start=True, stop=True)
                        sq = btmp.tile([P, T], F32, name="sqq")
                        nc.scalar.activation(sq[:], ps2[:], AF.Sqrt,
                                             scale=1.0 / HD, bias=eps_t[:])
                        rq = btmp.tile([P, T], F32, name="rqq")
                        nc.vector.reciprocal(rq[:], sq[:])
                        qn = btmp.tile([P, T], F32R, name="qn")
                        nc.vector.tensor_mul(qn[:], ps[:], rq[:])
                        psr = psR.tile([P, T], F32, name="rot")
                        nc.tensor.matmul(psr[:], rotp_t[:], qn[:],
                                         start=True, stop=True)
                        t1 = btmp.tile([P, T], F32, name="t1")
                        nc.gpsimd.tensor_mul(t1[:], qn[:], cos_ap)
                        t2 = btmp.tile([P, T], F32, name="t2")
                        nc.vector.tensor_mul(t2[:], psr[:], sin_ap)
                        nc.vector.tensor_add(out_tile, t1[:], t2[:])

                    def proj_ps(w_dram, o, xn_c):
                        wgt = wstr.tile([P, HT, P], F32R, name="wqg")
                        nc.sync.dma_start(wgt[:], w_dram[o])
                        ps = psA.tile([P, T], F32, name="proj")
                        for h in range(HT):
                            nc.tensor.matmul(ps[:], wgt[:, h, :], xn_c[h][:],
                                             start=(h == 0),
                                             stop=(h == HT - 1))
                        return ps

                    # chunk 0: local tokens (q, z, k half, v half)
                    with tc.tile_pool(name="xn0", bufs=HT) as xn0:
                        xn_c = rms_xn(0, xn0)
                        v_proj(0, xn_c)
                        for o in range(NH):
                            ps = proj_ps(wq_d, o, xn_c)
                            qk_pipeline(ps[:], qr_t[o][:],
                                        cosq_t[:], sinq_t[:])
                        for kv in range(NKV):
                            ps = proj_ps(wk_d, kv, xn_c)
                            qk_pipeline(ps[:], kr_t[kv][:, ts(0, T)],
                                        cosk_t[:, ts(0, T)],
                                        sink_t[:, ts(0, T)])
                        for o in range(NH):
                            ps = proj_ps(wz_d, o, xn_c)
                            nc.scalar.activation(sz_t[o][:], ps[:],
                                                 AF.Sigmoid)

                    # chunk 1: remote-half tokens (k and v only)
                    with tc.tile_pool(name="xn1", bufs=HT) as xn1:
                        xn_c = rms_xn(1, xn1)
                        v_proj(1, xn_c)
                        for kv in range(NKV):
                            ps = proj_ps(wk_d, kv, xn_c)
                            qk_pipeline(ps[:], kr_t[kv][:, ts(1, T)],
                                        cosk_t[:, ts(1, T)],
                                        sink_t[:, ts(1, T)])

                # ---- Phase C: attention ----
                with tc.tile_pool(name="gated", bufs=NH) as gpool:
                    gated_t = []
                    with tc.tile_pool(name="mask", bufs=1) as mpool, \
                         tc.tile_pool(name="probs", bufs=4) as ppool, \
                         tc.tile_pool(name="ctmp", bufs=2) as ctmp, \
                         tc.tile_pool(name="psSc", bufs=2,
                                      space="PSUM") as psSc, \
                         tc.tile_pool(name="psAt", bufs=2,
                                      space="PSUM") as psAt, \
                         tc.tile_pool(name="psSm", bufs=2,
                                      space="PSUM") as psSm, \
                         tc.tile_pool(name="psBc", bufs=2,
                                      space="PSUM") as psBc:

                        maskl_t = mpool.tile([P, 4, T], F32, name="maskl")
                        nc.sync.dma_start(maskl_t[:], maskl_d[:])
                        biasr_t = mpool.tile([P, 4], F32, name="biasr")
                        nc.sync.dma_start(biasr_t[:], biasr_d[:])

                        for o in range(NH):
                            kv = o // NKV
                            ps_att = psAt.tile([P, T], F32, name="att")
                            ps_sum = psSm.tile([1, T], F32, name="sum")
                            for j in range(NKB):
                                ps_sc = psSc.tile([P, T], F32, name="sc")
                                nc.tensor.matmul(ps_sc[:],
                                                 kr_t[kv][:, ts(j, P)],
                                                 qr_t[o][:],
                                                 start=True, stop=True)
                                probs = ppool.tile([P, T], F32R,
                                                   name="probs")
                                if j < 4:
                                    nc.scalar.activation(probs[:], ps_sc[:],
                                                         AF.Exp)
                                    nc.vector.tensor_mul(probs[:], probs[:],
                                                         maskl_t[:, j, :])
                                else:
                                    nc.scalar.activation(
                                        probs[:], ps_sc[:], AF.Exp,
                                        bias=biasr_t[:, ts(j - 4, 1)])
                                nc.tensor.matmul(ps_att[:],
                                                 v_t[j][:, ts(kv, P)],
                                                 probs[:], start=(j == 0),
                                                 stop=(j == NKB - 1))
                                nc.tensor.matmul(ps_sum[:], ones_t[:, 0:1],
                                                 probs[:], start=(j == 0),
                                                 stop=(j == NKB - 1))
                            rec = ctmp.tile([1, T], F32R, name="rec")
                            with nc.allow_low_precision("f32r = f32 bits"):
                                nc.vector.reciprocal(rec[:], ps_sum[:])
                            ps_bc = psBc.tile([P, T], F32, name="bc")
                            nc.tensor.matmul(ps_bc[:], ones_t[0:1, :],
                                             rec[:], start=True, stop=True)
                            recrep = ctmp.tile([P, T], F32, name="recrep")
                            nc.scalar.copy(recrep[:], ps_bc[:])
                            t1 = ctmp.tile([P, T], F32, name="ct1")
                            nc.vector.tensor_mul(t1[:], ps_att[:],
                                                 recrep[:])
                            g = gpool.tile([P, T], F32R, name="gated")
                            nc.vector.tensor_mul(g[:], t1[:], sz_t[o][:])
                            gated_t.append(g)

                    # ---- Phase D: o_proj + residual -> x2 scratch ----
                    with tc.tile_pool(name="wostr", bufs=2) as wostr, \
                         tc.tile_pool(name="rtmp", bufs=2) as rtmp, \
                         tc.tile_pool(name="x2w", bufs=3) as x2w, \
                         tc.tile_pool(name="psO", bufs=2,
                                      space="PSUM") as psO:
                        for hp in range(HT):
                            wgt = wostr.tile([P, NH, P], F32R, name="wog")
                            nc.sync.dma_start(wgt[:], wo_d[hp])
                            ps = psO.tile([P, T], F32, name="ops")
                            for o in range(NH):
                                nc.tensor.matmul(ps[:], wgt[:, o, :],
                                                 gated_t[o][:],
                                                 start=(o == 0),
                                                 stop=(o == NH - 1))
                            rx = rtmp.tile([P, T], F32, name="resid")
                            nc.sync.dma_start(rx[:], xt_d[ts(hp, P), 0:T])
                            x2t = x2w.tile([P, T], F32, name="x2t")
                            nc.vector.tensor_add(x2t[:], ps[:], rx[:])
                            nc.sync.dma_start(x2_d[ts(hp, P), :], x2t[:])

            # ============ MLP half: phases E-G ============
            with tc.tile_pool(name="mm", bufs=FT // 2) as mpool2, \
                 tc.tile_pool(name="h2", bufs=HT) as h2_pool:

                h2_t = [h2_pool.tile([P, T], F32R, name="h2")
                        for _ in range(HT)]

                # ---- Phase E: post rmsnorm (x2 from scratch DRAM) ----
                with tc.tile_pool(name="ea", bufs=3) as ea_pool, \
                     tc.tile_pool(name="eb", bufs=3) as eb_pool, \
                     tc.tile_pool(name="etmp", bufs=2) as etmp, \
                     tc.tile_pool(name="psE", bufs=1, space="PSUM") as psE:
                    ps = psE.tile([P, T], F32, name="essq")
                    for h in range(HT):
                        xf = ea_pool.tile([P, T], F32, name="ea")
                        nc.sync.dma_start(xf[:], x2_d[ts(h, P), :])
                        xsq = etmp.tile([P, T], F32R, name="exsq")
                        nc.scalar.activation(xsq[:], xf[:], AF.Square)
                        nc.tensor.matmul(ps[:], ones_t[:], xsq[:],
                                         start=(h == 0), stop=(h == HT - 1))
                    sq = etmp.tile([P, T], F32, name="esq")
                    nc.scalar.activation(sq[:], ps[:], AF.Sqrt,
                                         scale=1.0 / H, bias=eps_t[:])
                    rstd = etmp.tile([P, T], F32, name="erstd")
                    nc.vector.reciprocal(rstd[:], sq[:])
                    for h in range(HT):
                        xf2 = eb_pool.tile([P, T], F32, name="eb")
                        nc.sync.dma_start(xf2[:], x2_d[ts(h, P), :])
                        nc.vector.tensor_mul(h2_t[h][:], xf2[:], rstd[:])

                # ---- Phase F+G: gate/up/silu/down in two f-halves ----
                FH = FT // 2
                with tc.tile_pool(name="wgstr", bufs=2) as wgstr, \
                     tc.tile_pool(name="wustr", bufs=2) as wustr, \
                     tc.tile_pool(name="wdstr", bufs=2) as wdstr, \
                     tc.tile_pool(name="ftmp", bufs=2) as ftmp, \
                     tc.tile_pool(name="gtmp", bufs=3) as gtmp, \
                     tc.tile_pool(name="psG", bufs=2, space="PSUM") as psG, \
                     tc.tile_pool(name="psU", bufs=2, space="PSUM") as psU, \
                     tc.tile_pool(name="psD", bufs=2, space="PSUM") as psD:
                    for fh in range(2):
                        m_t = []
                        for fi in range(FH):
                            f = fh * FH + fi
                            wgt = wgstr.tile([P, HT, P], F32R, name="wgg")
                            nc.sync.dma_start(wgt[:], wg_d[f])
                            wut = wustr.tile([P, HT, P], F32R, name="wug")
                            nc.sync.dma_start(wut[:], wu_d[f])
                            psg = psG.tile([P, T], F32, name="gps")
                            psu = psU.tile([P, T], F32, name="ups")
                            for h in range(HT):
                                nc.tensor.matmul(psg[:], wgt[:, h, :],
                                                 h2_t[h][:], start=(h == 0),
                                                 stop=(h == HT - 1))
                            for h in range(HT):
                                nc.tensor.matmul(psu[:], wut[:, h, :],
                                                 h2_t[h][:], start=(h == 0),
                                                 stop=(h == HT - 1))
                            sg0 = ftmp.tile([P, T], F32, name="sgm")
                            nc.scalar.activation(sg0[:], psg[:], AF.Sigmoid)
                            sg = ftmp.tile([P, T], F32, name="silu")
                            nc.vector.tensor_mul(sg[:], psg[:], sg0[:])
                            mt = mpool2.tile([P, T], F32R, name="mt")
                            nc.vector.tensor_mul(mt[:], psu[:], sg[:])
                            m_t.append(mt)
                        # down projection partial over this f-half
                        for h in range(HT):
                            ps = psD.tile([P, T], F32, name="dps")
                            for q4 in range(2):
                                wdt = wdstr.tile([P, FH // 2, P], F32R,
                                                 name="wdg")
                                nc.sync.dma_start(
                                    wdt[:],
                                    wd_d[h, :,
                                         ts(fh * 2 + q4, FH // 2), :])
                                for fi in range(FH // 2):
                                    fidx = q4 * (FH // 2) + fi
                                    nc.tensor.matmul(
                                        ps[:], wdt[:, fi, :], m_t[fidx][:],
                                        start=(fidx == 0),
                                        stop=(fidx == FH - 1))
                            prev = gtmp.tile([P, T], F32, name="gprev")
                            nc.sync.dma_start(
                                prev[:],
                                x2_d[ts(h, P), :] if fh == 0
                                else out_d[ts(h, P), :])
                            outt = gtmp.tile([P, T], F32, name="gout")
                            nc.vector.tensor_add(outt[:], ps[:], prev[:])
                            nc.sync.dma_start(out_d[ts(h, P), :], outt[:])

    nc.compile()
    _BUILD_CACHE["nc"] = nc
    return nc


def _prep_core_inputs(inputs):
    """Host-side preprocessing: fold norms/scales into weights and tables,
    transpose + tile weights for contiguous DMA, build per-core in_maps."""
    f32 = np.float32
    x = np.asarray(inputs["x"], f32)
    in_ln_w = np.asarray(inputs["in_ln_w"], f32)
    post_ln_w = np.asarray(inputs["post_ln_w"], f32)
    qn_w = np.asarray(inputs["qn_w"], f32)
    kn_w = np.asarray(inputs["kn_w"], f32)

    s_in = (1.0 + in_ln_w)[:, None]       # [H, 1] scale on contraction dim
    s_post = (1.0 + post_ln_w)[:, None]

    def tile_lhsT(wT, n_out_tiles):
        # wT: [K_total, M_total] -> [o_tile, p(=K within), i(=K tile), c]
        kt = wT.shape[0] // P
        a = np.ascontiguousarray(
            wT.reshape(kt, P, n_out_tiles, P).transpose(2, 1, 0, 3))
        return a.astype(f32)

    wq = tile_lhsT(np.asarray(inputs["Wq"], f32).T * s_in, NH)
    wk = tile_lhsT(np.asarray(inputs["Wk"], f32).T * s_in, NKV)
    wz = tile_lhsT(np.asarray(inputs["Wz"], f32).T * s_in, NH)
    wo = tile_lhsT(np.asarray(inputs["Wo"], f32).T, HT)
    wg = tile_lhsT(np.asarray(inputs["Wg"], f32).T * s_post, FT)
    wu = tile_lhsT(np.asarray(inputs["Wu"], f32).T * s_post, FT)
    wd = tile_lhsT(np.asarray(inputs["Wd"], f32).T, HT)
    wv = np.ascontiguousarray(
        (np.asarray(inputs["Wv"], f32).T * s_in).reshape(HT, P, NKV * HD)
    ).astype(f32)

    # rope tables
    inv_freq = 1.0 / (10000.0 ** (np.arange(0, HD, 2, dtype=f32) / HD))
    t = np.arange(S, dtype=f32)
    freqs = t[:, None] * inv_freq[None, :]
    emb = np.concatenate([freqs, freqs], axis=-1)     # [S, HD]
    cos_all, sin_all = np.cos(emb), np.sin(emb)
    rolled_q = np.roll(1.0 + qn_w, -64)
    rolled_k = np.roll(1.0 + kn_w, -64)
    inv_sqrt_hd = 1.0 / np.sqrt(np.float32(HD))

    ones = np.ones((P, P), f32)
    rotp = np.zeros((P, P), f32)
    for i in range(64):
        rotp[i + 64, i] = -1.0
        rotp[i, i + 64] = 1.0

    qk = np.arange(T)[None, :]            # query col
    kk = np.arange(P)[:, None]            # key row within block
    maskl = np.zeros((P, 4, T), f32)
    for j in range(4):
        maskl[:, j, :] = (P * j + kk <= qk).astype(f32)

    in_maps = []
    for c in range(NCORES):
        b, half = c // 2, c % 2
        p0 = half * T
        pos = np.concatenate([np.arange(p0, p0 + T),
                              np.arange(T - p0, 2 * T - p0)])  # local first
        xt = np.ascontiguousarray(x[b][pos].T)                 # [H, S]
        pos_q = pos[:T]
        cosq = np.ascontiguousarray(
            (cos_all[pos_q] * (1.0 + qn_w)[None, :] * inv_sqrt_hd).T)
        sinq = np.ascontiguousarray(
            (sin_all[pos_q] * rolled_q[None, :] * inv_sqrt_hd).T)
        cosk = np.ascontiguousarray((cos_all[pos] * (1.0 + kn_w)[None, :]).T)
        sink = np.ascontiguousarray((sin_all[pos] * rolled_k[None, :]).T)
        biasr = np.full((P, 4), 0.0 if half == 1 else -1e30, f32)
        in_maps.append({
            "xt": xt, "wq": wq, "wk": wk, "wv": wv, "wz": wz, "wo": wo,
            "wg": wg, "wu": wu, "wd": wd,
            "cosq": cosq, "sinq": sinq, "cosk": cosk, "sink": sink,
            "maskl": maskl, "biasr": biasr, "ones": ones, "rotp": rotp,
        })
    return in_maps


def kernel(**inputs):
    nc = _build_program()
    in_maps = _prep_core_inputs(inputs)
    res = run_bass_kernel_spmd(nc, in_maps, list(range(NCORES)))
    out = np.empty((B, S, H), np.float32)
    for c in range(NCORES):
        b, half = c // 2, c % 2
        out[b, half * T:(half + 1) * T, :] = res.results[c]["outT"].T
    return out
